# revision 31
# baseline (speedup 1.0000x reference)
"""ANFIS layer kernel for 8 Trainium2 NeuronCores (data-parallel over batch).

Math (reference):
  mu[b,i,t]   = exp(-(x[b,i]-c[i,t])^2 / (2 s[i,t]^2)),  s = |sigmas|+1e-6
  w[b,r]      = prod_i mu[b,i,RULE_IDX[r,i]]   (RULE_IDX = full cross product)
  out[b]      = sum_r w[b,r] f[b,r] / (sum_r w[b,r] + 1e-8),  f = [x,1] @ C^T

Factorization used here (r = A*256 + B, A=(t0,t1) in 16, B=(t2..t5) in 256):
  sum_r w[b,r]        = prod_i sum_t mu[b,i,t]                (product of sums)
  sum_r w[b,r] f[b,r] = sum_{A,B,j} P01[b,A] P2345[b,B] xa[b,j] C[A,B,j]
                      = sum_{A,j} (P01 (x) xa)[b,(A,j)] * (P2345[b,:] @ C2)[b,(A,j)]
  with C2[B,(A,j)] = consequents[A*256+B, j].

Device mapping per core (4096 batch rows, packed 32 rows per partition):
  - log-memberships lmu = -(x-c)^2/(2 s^2) built with packed vector ops
  - P2345[b,B] = exp(lP23[b,B//16] + lP45[b,B%16]) computed on TensorE:
    transpose the 32 stacked log-pair columns, matmul with a 0/1
    replication matrix RT[32,256], exp on ScalarE (PSUM->SBUF)
  - G = P2345 @ C2 via 2 PSUM-accumulating matmuls (K=128 each)
  - numerator via fused multiply-reduce against Q = P01 (x) x_aug
"""

import os
import sys

import numpy as np

if "/opt/trn_rl_repo" not in sys.path:
    sys.path.insert(0, "/opt/trn_rl_repo")

import concourse.bass as bass
import concourse.mybir as mybir
from concourse import tile
from concourse import bass_utils

F32 = mybir.dt.float32
BF16 = mybir.dt.bfloat16
AF = mybir.ActivationFunctionType
ALU = mybir.AluOpType
AX = mybir.AxisListType

NCORES = 8
B_FULL = 32768
BC = B_FULL // NCORES  # 4096 rows per core
P = 128                # partitions
RPP = BC // P          # 32 batch rows packed per partition
NI = 6                 # inputs
NT = 4                 # terms per input
NA = 16                # left modes  (t0,t1)
NB = 256               # right modes (t2..t5)
NJ = NI + 1            # augmented input width
NQ = NA * NJ           # 112


def _v(ap, ap_list, extra_offset=0):
    """Build a raw AP view with an explicit [step, count] list."""
    return bass.AP(ap.tensor, ap.offset + extra_offset, [list(d) for d in ap_list])


def split_multi_waits(nc):
    """TRN2 instructions hold at most one sync-wait; walrus rejects more.

    Tile emits multi-wait instructions, so hoist all but the last wait into
    single-wait EventSemaphore instructions on the same engine (engine
    program order preserves semantics).
    """
    for f in nc.m.functions:
        for blk in f.blocks:
            i = 0
            while i < len(blk.instructions):
                inst = blk.instructions[i]
                si = inst.sync_info
                if (
                    si is not None
                    and si.on_wait
                    and len(si.on_wait) > 1
                    and inst.engine != mybir.EngineType.Unassigned
                ):
                    waits = list(si.on_wait)
                    for w in waits[:-1]:
                        ev = mybir.InstEventSemaphore(
                            name=nc.get_next_instruction_name(), ins=[], outs=[]
                        )
                        ev.engine = inst.engine
                        ev.sync_info = mybir.SyncInfo(on_wait=[w], on_update=[])
                        nc.register_instruction(ev)
                        blk.instructions.insert(i, ev)
                        i += 1
                    inst.sync_info = mybir.SyncInfo(
                        on_wait=[waits[-1]], on_update=list(si.on_update)
                    )
                i += 1
    return nc


def build_kernel():
    nc = bass.Bass(target_bir_lowering=False)

    x_e = nc.declare_dram_parameter("x", [BC, NI], F32, isOutput=False)
    c_e = nc.declare_dram_parameter("centers", [NI, NT], F32, isOutput=False)
    s_e = nc.declare_dram_parameter("sigmas", [NI, NT], F32, isOutput=False)
    q_e = nc.declare_dram_parameter("consequents", [4096, NJ], F32, isOutput=False)
    rt_e = nc.declare_dram_parameter("rt", [P, NB], BF16, isOutput=False)
    id_e = nc.declare_dram_parameter("ident", [P, P], BF16, isOutput=False)
    o_e = nc.declare_dram_parameter("out", [BC, 1], F32, isOutput=True)

    with tile.TileContext(nc) as tc:
        with (
            tc.tile_pool(name="const", bufs=1) as cp,
            tc.tile_pool(name="lppT", bufs=4) as lp_pool,
            tc.tile_pool(name="p45", bufs=3) as p45_pool,
            tc.tile_pool(name="scr", bufs=4) as scr_pool,
            tc.tile_pool(name="ptt", bufs=2, space="PSUM") as ptt_pool,
            tc.tile_pool(name="plw", bufs=2, space="PSUM") as plw_pool,
            tc.tile_pool(name="pg", bufs=2, space="PSUM") as pg_pool,
        ):
            # ---------------- inputs -> SBUF ----------------
            # params first (the membership chain needs them), broadcast to all
            # 128 partitions with a 0-step source AP; then x; then the
            # later-needed constants.
            cab = cp.tile([P, 48], F32)  # [cB(24) | aB(24)] replicated
            sB = cp.tile([P, 24], F32)
            nc.sync.dma_start(
                _v(cab[:, :], [[48, P], [1, 24]]), _v(c_e.ap(), [[0, P], [1, 24]])
            )
            nc.sync.dma_start(
                _v(sB[:, :], [[24, P], [1, 24]]), _v(s_e.ap(), [[0, P], [1, 24]])
            )
            xs = cp.tile([P, RPP * NI], F32)  # x packed: [p, r, i]
            nc.sync.dma_start(xs[:, :], x_e.ap().rearrange("(p r) i -> p (r i)", p=P))

            idn = cp.tile([P, P], BF16)
            nc.sync.dma_start(idn[:, :], id_e[:, :])
            rt = cp.tile([P, NB], BF16)
            nc.sync.dma_start(rt[:, :], rt_e[:, :])

            # C2[B, (A,j)] = consequents[A*256+B, j]; chunk k holds B in [128k, 128k+128)
            c2f = cp.tile([P, 2 * NQ], F32)
            cap = [[NJ, P], [NB * NJ, NA], [1, NJ]]
            nc.sync.dma_start(c2f[:, 0:NQ], _v(q_e.ap(), cap))
            nc.sync.dma_start(c2f[:, NQ : 2 * NQ], _v(q_e.ap(), cap, extra_offset=P * NJ))
            # aB = -1/(2 s^2)  (sigmas in [0.5, 1.5]: (|s|+1e-6)^2 ~= s^2)
            cab_ap = cab[:, :]
            nc.vector.tensor_mul(sB[:, :], sB[:, :], sB[:, :])
            nc.vector.reciprocal(_v(cab_ap, [[48, P], [1, 24]], 24), sB[:, :])
            nc.vector.tensor_scalar_mul(
                _v(cab_ap, [[48, P], [1, 24]], 24),
                _v(cab_ap, [[48, P], [1, 24]], 24),
                -0.5,
            )
            # cB = cab[:, 0:24] ; aB = cab[:, 24:48]

            # ---------------- membership prologue, split in row-halves so the
            # TensorEngine groups can start while the second half computes.
            # The lpp chain is emitted first (it gates the TensorE groups);
            # mu/P01/Q/e1 follow once both halves' logs are in flight. ----
            d = cp.tile([P, RPP * 24], F32)
            sq = cp.tile([P, RPP * 24], F32)
            lmu = cp.tile([P, RPP * 24], F32)
            mu = cp.tile([P, RPP * 24], F32)
            lpp = cp.tile([P, RPP * 32], BF16)
            p01 = cp.tile([P, RPP * NA], F32)
            q = cp.tile([P, RPP * NQ], BF16)
            e1 = cp.tile([P, RPP * NI], F32)
            xa = cp.tile([P, RPP * NJ], F32)
            d_ap = d[:, :]
            mu_ap = mu[:, :]
            lpp_ap = lpp[:, :]
            lmu_ap = lmu[:, :]
            p01_ap = p01[:, :]
            e1_ap = e1[:, :]
            xa_ap = xa[:, :]
            xs_ap = xs[:, :]
            HR = RPP // 2  # rows per half
            NCH = 4        # chunks for the log chain (first groups start early)
            CR = RPP // NCH
            for h in range(NCH):
                o24 = h * CR * 24
                o32 = h * CR * 32
                o6 = h * CR * NI
                # d[p,r,i,t] = x[p,r,i] - c[i,t] ; lmu = -(d*d) / (2 s^2)
                nc.vector.tensor_sub(
                    _v(d_ap, [[RPP * 24, P], [24, CR], [4, NI], [1, NT]], o24),
                    _v(xs_ap, [[RPP * NI, P], [NI, CR], [1, NI], [0, NT]], o6),
                    _v(cab_ap, [[48, P], [0, CR], [4, NI], [1, NT]]),
                )
                nc.vector.tensor_mul(
                    _v(sq[:, :], [[RPP * 24, P], [1, CR * 24]], o24),
                    _v(d_ap, [[RPP * 24, P], [1, CR * 24]], o24),
                    _v(d_ap, [[RPP * 24, P], [1, CR * 24]], o24),
                )
                nc.vector.tensor_mul(
                    _v(lmu_ap, [[RPP * 24, P], [1, CR * 24]], o24),
                    _v(sq[:, :], [[RPP * 24, P], [1, CR * 24]], o24),
                    _v(cab_ap, [[48, P], [0, CR], [1, 24]], 24),
                )
                # lPP pair-log sums (feeds the TensorE groups)
                nc.vector.tensor_add(
                    _v(lpp_ap, [[RPP * 32, P], [32, CR], [4, 4], [1, 4]], o32),
                    _v(lmu_ap, [[RPP * 24, P], [24, CR], [1, 4], [0, 4]], o24 + 8),
                    _v(lmu_ap, [[RPP * 24, P], [24, CR], [0, 4], [1, 4]], o24 + 12),
                )
                nc.vector.tensor_add(
                    _v(lpp_ap, [[RPP * 32, P], [32, CR], [4, 4], [1, 4]], o32 + 16),
                    _v(lmu_ap, [[RPP * 24, P], [24, CR], [1, 4], [0, 4]], o24 + 16),
                    _v(lmu_ap, [[RPP * 24, P], [24, CR], [0, 4], [1, 4]], o24 + 20),
                )
                # mu = exp(lmu) (ACT, off the DVE critical path)
                nc.scalar.activation(
                    _v(mu_ap, [[RPP * 24, P], [1, CR * 24]], o24),
                    _v(lmu_ap, [[RPP * 24, P], [1, CR * 24]], o24),
                    AF.Exp,
                )

            # late constants: c2 cast and x_aug (needed by GMM / Q)
            c2 = cp.tile([P, 2 * NQ], BF16)
            nc.vector.tensor_copy(c2[:, :], c2f[:, :])
            nc.vector.tensor_copy(
                _v(xa_ap, [[RPP * NJ, P], [NJ, RPP], [1, NI]]),
                _v(xs_ap, [[RPP * NI, P], [NI, RPP], [1, NI]]),
            )
            nc.vector.memset(_v(xa_ap, [[RPP * NJ, P], [NJ, RPP], [1, 1]], 6), 1.0)

            for h in range(2):
                o24 = h * HR * 24
                o6 = h * HR * NI
                o7 = h * HR * NJ
                oA = h * HR * NA
                oQ = h * HR * NQ
                nc.vector.tensor_mul(
                    _v(p01_ap, [[RPP * NA, P], [NA, HR], [NT, NT], [1, NT]], oA),
                    _v(mu_ap, [[RPP * 24, P], [24, HR], [1, NT], [0, NT]], o24 + 0),
                    _v(mu_ap, [[RPP * 24, P], [24, HR], [0, NT], [1, NT]], o24 + 4),
                )
                # Q = P01 (x) x_aug
                nc.vector.tensor_mul(
                    _v(q[:, :], [[RPP * NQ, P], [NQ, HR], [NJ, NA], [1, NJ]], oQ),
                    _v(p01_ap, [[RPP * NA, P], [NA, HR], [1, NA], [0, NJ]], oA),
                    _v(xa_ap, [[RPP * NJ, P], [NJ, HR], [0, NA], [1, NJ]], o7),
                )
                # denominator partial: e1[p,r,i] = sum_t mu
                nc.vector.tensor_reduce(
                    _v(e1_ap, [[RPP * NI, P], [NI, HR], [1, NI]], o6),
                    _v(mu_ap, [[RPP * 24, P], [24, HR], [4, NI], [1, NT]], o24),
                    axis=AX.X,
                    op=ALU.add,
                )

            # ---------------- denominator: prod_i of the 6 sums ----------------
            p3 = cp.tile([P, RPP * 3], F32)
            nc.vector.tensor_mul(
                _v(p3[:, :], [[RPP * 3, P], [3, RPP], [1, 3]]),
                _v(e1_ap, [[RPP * NI, P], [NI, RPP], [1, 3]]),
                _v(e1_ap, [[RPP * NI, P], [NI, RPP], [1, 3]], 3),
            )
            p3_ap = p3[:, :]
            den1 = cp.tile([P, RPP], F32)
            den2 = cp.tile([P, RPP], F32)
            nc.vector.tensor_mul(
                den1[:, :],
                _v(p3_ap, [[RPP * 3, P], [3, RPP]], 0),
                _v(p3_ap, [[RPP * 3, P], [3, RPP]], 1),
            )
            nc.vector.tensor_mul(
                den2[:, :],
                den1[:, :],
                _v(p3_ap, [[RPP * 3, P], [3, RPP]], 2),
            )
            dene = cp.tile([P, RPP], F32)
            recip = cp.tile([P, RPP], F32)
            nc.vector.tensor_scalar_add(dene[:, :], den2[:, :], 1e-8)
            nc.vector.reciprocal(recip[:, :], dene[:, :])

            # ---------------- per packed-row pass (groups of 4 rows) ----------------
            num = cp.tile([P, RPP], F32)
            GR = 4  # rows per group
            for g in range(RPP // GR):
                # 4 transposes -> one [32, 4*128] psum tile: cols = (rr, b)
                ptt4 = ptt_pool.tile([32, GR * P], BF16)
                for rr in range(GR):
                    r = g * GR + rr
                    nc.tensor.transpose(
                        ptt4[:, rr * P : (rr + 1) * P],
                        lpp[:, r * 32 : (r + 1) * 32],
                        idn[:, :],
                    )
                lppT4 = lp_pool.tile([32, GR * P], BF16)
                nc.scalar.copy(lppT4[:, :], ptt4[:, :])

                # Kron-expansion matmuls, N = GR*128 covering GR rows at once
                NW = GR * P
                plw4 = plw_pool.tile([P, 2 * NW], F32)
                for ch in range(2):
                    nc.tensor.matmul(
                        plw4[:, ch * NW : ch * NW + NW],
                        rt[0:32, ch * 128 : (ch + 1) * 128],
                        lppT4[:, :],
                        start=True,
                        stop=True,
                    )
                p45g = p45_pool.tile([P, 2 * NW], BF16)
                for eh in range(2 * NW // 512):
                    nc.scalar.activation(
                        p45g[:, eh * 512 : (eh + 1) * 512],
                        plw4[:, eh * 512 : (eh + 1) * 512],
                        AF.Exp,
                    )

                pg4 = pg_pool.tile([P, GR * NQ], F32, tag="pg")
                for rr in range(GR):
                    nc.tensor.matmul(
                        pg4[:, rr * NQ : (rr + 1) * NQ],
                        p45g[:, rr * P : (rr + 1) * P],
                        c2[:, 0:NQ],
                        start=True,
                        stop=False,
                    )
                    nc.tensor.matmul(
                        pg4[:, rr * NQ : (rr + 1) * NQ],
                        p45g[:, NW + rr * P : NW + (rr + 1) * P],
                        c2[:, NQ : 2 * NQ],
                        start=False,
                        stop=True,
                    )
                for rr in range(GR):
                    r = g * GR + rr
                    scr = scr_pool.tile([P, NQ], F32)
                    nc.vector.scalar_tensor_tensor(
                        out=scr[:, :],
                        in0=pg4[:, rr * NQ : (rr + 1) * NQ],
                        scalar=1.0,
                        in1=q[:, r * NQ : (r + 1) * NQ],
                        op0=ALU.mult,
                        op1=ALU.mult,
                        accum_out=num[:, r : r + 1],
                    )

            # ---------------- out = num * recip, store in row-halves ----------
            res = cp.tile([P, RPP], F32)
            o_v = o_e.ap().rearrange("(p r) o -> p (r o)", p=P)
            for h in range(2):
                oR = h * HR
                nc.vector.tensor_mul(
                    res[:, oR : oR + HR], num[:, oR : oR + HR], recip[:, oR : oR + HR]
                )
                nc.sync.dma_start(
                    _v(o_v, [[RPP, P], [1, HR]], oR), res[:, oR : oR + HR]
                )

    return nc


def make_rt():
    rtm = np.zeros((32, NB), dtype=np.float32)
    for bm in range(NB):
        rtm[bm // 16, bm] = 1.0
        rtm[16 + bm % 16, bm] = 1.0
    return np.tile(rtm, (4, 1))  # replicated at partition bases 0/32/64/96


_CACHE = {}


def _get_nc():
    if "nc" not in _CACHE:
        _CACHE["nc"] = split_multi_waits(build_kernel())
    return _CACHE["nc"]


def make_in_maps(x, centers, sigmas, consequents):
    x = np.ascontiguousarray(x, dtype=np.float32)
    centers = np.ascontiguousarray(centers, dtype=np.float32)
    sigmas = np.ascontiguousarray(sigmas, dtype=np.float32)
    consequents = np.ascontiguousarray(consequents, dtype=np.float32)
    import ml_dtypes

    rtm = make_rt().astype(ml_dtypes.bfloat16)
    idn = np.eye(P, dtype=ml_dtypes.bfloat16)
    in_maps = []
    for i in range(NCORES):
        in_maps.append(
            {
                "x": x[i * BC : (i + 1) * BC],
                "centers": centers,
                "sigmas": sigmas,
                "consequents": consequents,
                "rt": rtm,
                "ident": idn,
            }
        )
    return in_maps


def kernel(x, centers, sigmas, consequents):
    nc = _get_nc()
    in_maps = make_in_maps(x, centers, sigmas, consequents)
    res = bass_utils.run_bass_kernel_spmd(nc, in_maps, core_ids=list(range(NCORES)))
    outs = [res.results[i]["out"] for i in range(NCORES)]
    return np.concatenate(outs, axis=0).astype(np.float32)


if __name__ == "__main__":
    rng = np.random.default_rng(0)
    x = rng.standard_normal((B_FULL, NI), dtype=np.float32)
    centers = rng.standard_normal((NI, NT), dtype=np.float32)
    sigmas = rng.uniform(0.5, 1.5, (NI, NT)).astype(np.float32)
    consequents = rng.standard_normal((4096, NJ), dtype=np.float32)
    out = kernel(x, centers, sigmas, consequents)
    print(out.shape, out.dtype, out[:4, 0])


# revision 32
# speedup vs baseline: 1.0032x; 1.0032x over previous
"""ANFIS layer kernel for 8 Trainium2 NeuronCores (data-parallel over batch).

Math (reference):
  mu[b,i,t]   = exp(-(x[b,i]-c[i,t])^2 / (2 s[i,t]^2)),  s = |sigmas|+1e-6
  w[b,r]      = prod_i mu[b,i,RULE_IDX[r,i]]   (RULE_IDX = full cross product)
  out[b]      = sum_r w[b,r] f[b,r] / (sum_r w[b,r] + 1e-8),  f = [x,1] @ C^T

Factorization used here (r = A*256 + B, A=(t0,t1) in 16, B=(t2..t5) in 256):
  sum_r w[b,r]        = prod_i sum_t mu[b,i,t]                (product of sums)
  sum_r w[b,r] f[b,r] = sum_{A,B,j} P01[b,A] P2345[b,B] xa[b,j] C[A,B,j]
                      = sum_{A,j} (P01 (x) xa)[b,(A,j)] * (P2345[b,:] @ C2)[b,(A,j)]
  with C2[B,(A,j)] = consequents[A*256+B, j].

Device mapping per core (4096 batch rows, packed 32 rows per partition):
  - log-memberships lmu = -(x-c)^2/(2 s^2) built with packed vector ops
  - P2345[b,B] = exp(lP23[b,B//16] + lP45[b,B%16]) computed on TensorE:
    transpose the 32 stacked log-pair columns, matmul with a 0/1
    replication matrix RT[32,256], exp on ScalarE (PSUM->SBUF)
  - G = P2345 @ C2 via 2 PSUM-accumulating matmuls (K=128 each)
  - numerator via fused multiply-reduce against Q = P01 (x) x_aug
"""

import os
import sys

import numpy as np

if "/opt/trn_rl_repo" not in sys.path:
    sys.path.insert(0, "/opt/trn_rl_repo")

import concourse.bass as bass
import concourse.mybir as mybir
from concourse import tile
from concourse import bass_utils

F32 = mybir.dt.float32
BF16 = mybir.dt.bfloat16
AF = mybir.ActivationFunctionType
ALU = mybir.AluOpType
AX = mybir.AxisListType

NCORES = 8
B_FULL = 32768
BC = B_FULL // NCORES  # 4096 rows per core
P = 128                # partitions
RPP = BC // P          # 32 batch rows packed per partition
NI = 6                 # inputs
NT = 4                 # terms per input
NA = 16                # left modes  (t0,t1)
NB = 256               # right modes (t2..t5)
NJ = NI + 1            # augmented input width
NQ = NA * NJ           # 112


def _v(ap, ap_list, extra_offset=0):
    """Build a raw AP view with an explicit [step, count] list."""
    return bass.AP(ap.tensor, ap.offset + extra_offset, [list(d) for d in ap_list])


def split_multi_waits(nc):
    """TRN2 instructions hold at most one sync-wait; walrus rejects more.

    Tile emits multi-wait instructions, so hoist all but the last wait into
    single-wait EventSemaphore instructions on the same engine (engine
    program order preserves semantics).
    """
    for f in nc.m.functions:
        for blk in f.blocks:
            i = 0
            while i < len(blk.instructions):
                inst = blk.instructions[i]
                si = inst.sync_info
                if (
                    si is not None
                    and si.on_wait
                    and len(si.on_wait) > 1
                    and inst.engine != mybir.EngineType.Unassigned
                ):
                    waits = list(si.on_wait)
                    for w in waits[:-1]:
                        ev = mybir.InstEventSemaphore(
                            name=nc.get_next_instruction_name(), ins=[], outs=[]
                        )
                        ev.engine = inst.engine
                        ev.sync_info = mybir.SyncInfo(on_wait=[w], on_update=[])
                        nc.register_instruction(ev)
                        blk.instructions.insert(i, ev)
                        i += 1
                    inst.sync_info = mybir.SyncInfo(
                        on_wait=[waits[-1]], on_update=list(si.on_update)
                    )
                i += 1
    return nc


def build_kernel():
    nc = bass.Bass(target_bir_lowering=False)

    x_e = nc.declare_dram_parameter("x", [BC, NI], F32, isOutput=False)
    c_e = nc.declare_dram_parameter("centers", [NI, NT], F32, isOutput=False)
    s_e = nc.declare_dram_parameter("sigmas", [NI, NT], F32, isOutput=False)
    q_e = nc.declare_dram_parameter("consequents", [4096, NJ], F32, isOutput=False)
    rt_e = nc.declare_dram_parameter("rt", [P, NB], BF16, isOutput=False)
    id_e = nc.declare_dram_parameter("ident", [P, P], BF16, isOutput=False)
    o_e = nc.declare_dram_parameter("out", [BC, 1], F32, isOutput=True)

    with tile.TileContext(nc) as tc:
        with (
            tc.tile_pool(name="const", bufs=1) as cp,
            tc.tile_pool(name="lppT", bufs=4) as lp_pool,
            tc.tile_pool(name="p45", bufs=3) as p45_pool,
            tc.tile_pool(name="scr", bufs=4) as scr_pool,
            tc.tile_pool(name="ptt", bufs=2, space="PSUM") as ptt_pool,
            tc.tile_pool(name="plw", bufs=2, space="PSUM") as plw_pool,
            tc.tile_pool(name="pg", bufs=2, space="PSUM") as pg_pool,
        ):
            # ---------------- inputs -> SBUF ----------------
            # params first (the membership chain needs them), broadcast to all
            # 128 partitions with a 0-step source AP; then x; then the
            # later-needed constants.
            cab = cp.tile([P, 48], F32)  # [cB(24) | aB(24)] replicated
            sB = cp.tile([P, 24], F32)
            nc.sync.dma_start(
                _v(cab[:, :], [[48, P], [1, 24]]), _v(c_e.ap(), [[0, P], [1, 24]])
            )
            nc.sync.dma_start(
                _v(sB[:, :], [[24, P], [1, 24]]), _v(s_e.ap(), [[0, P], [1, 24]])
            )
            xs = cp.tile([P, RPP * NI], F32)  # x packed: [p, r, i]
            nc.sync.dma_start(xs[:, :], x_e.ap().rearrange("(p r) i -> p (r i)", p=P))

            idn = cp.tile([P, P], BF16)
            nc.sync.dma_start(idn[:, :], id_e[:, :])
            rt = cp.tile([P, NB], BF16)
            nc.sync.dma_start(rt[:, :], rt_e[:, :])

            # C2[B, (A,j)] = consequents[A*256+B, j]; chunk k holds B in [128k, 128k+128)
            c2f = cp.tile([P, 2 * NQ], F32)
            cap = [[NJ, P], [NB * NJ, NA], [1, NJ]]
            nc.sync.dma_start(c2f[:, 0:NQ], _v(q_e.ap(), cap))
            nc.sync.dma_start(c2f[:, NQ : 2 * NQ], _v(q_e.ap(), cap, extra_offset=P * NJ))
            # aB = -1/(2 s^2)  (sigmas in [0.5, 1.5]: (|s|+1e-6)^2 ~= s^2)
            cab_ap = cab[:, :]
            nc.vector.tensor_mul(sB[:, :], sB[:, :], sB[:, :])
            nc.vector.reciprocal(_v(cab_ap, [[48, P], [1, 24]], 24), sB[:, :])
            nc.vector.tensor_scalar_mul(
                _v(cab_ap, [[48, P], [1, 24]], 24),
                _v(cab_ap, [[48, P], [1, 24]], 24),
                -0.5,
            )
            # cB = cab[:, 0:24] ; aB = cab[:, 24:48]

            # ---------------- membership prologue, split in row-halves so the
            # TensorEngine groups can start while the second half computes.
            # The lpp chain is emitted first (it gates the TensorE groups);
            # mu/P01/Q/e1 follow once both halves' logs are in flight. ----
            d = cp.tile([P, RPP * 24], F32)
            sq = cp.tile([P, RPP * 24], F32)
            lmu = cp.tile([P, RPP * 24], F32)
            mu = cp.tile([P, RPP * 24], F32)
            lpp = cp.tile([P, RPP * 32], BF16)
            p01 = cp.tile([P, RPP * NA], F32)
            q = cp.tile([P, RPP * NQ], BF16)
            e1 = cp.tile([P, RPP * NI], F32)
            xa = cp.tile([P, RPP * NJ], F32)
            d_ap = d[:, :]
            mu_ap = mu[:, :]
            lpp_ap = lpp[:, :]
            lmu_ap = lmu[:, :]
            p01_ap = p01[:, :]
            e1_ap = e1[:, :]
            xa_ap = xa[:, :]
            xs_ap = xs[:, :]
            HR = RPP // 2  # rows per half
            NCH = 4        # chunks for the log chain (first groups start early)
            CR = RPP // NCH
            for h in range(NCH):
                o24 = h * CR * 24
                o32 = h * CR * 32
                o6 = h * CR * NI
                # d[p,r,i,t] = x[p,r,i] - c[i,t] ; lmu = -(d*d) / (2 s^2)
                nc.vector.tensor_sub(
                    _v(d_ap, [[RPP * 24, P], [24, CR], [4, NI], [1, NT]], o24),
                    _v(xs_ap, [[RPP * NI, P], [NI, CR], [1, NI], [0, NT]], o6),
                    _v(cab_ap, [[48, P], [0, CR], [4, NI], [1, NT]]),
                )
                nc.vector.tensor_mul(
                    _v(sq[:, :], [[RPP * 24, P], [1, CR * 24]], o24),
                    _v(d_ap, [[RPP * 24, P], [1, CR * 24]], o24),
                    _v(d_ap, [[RPP * 24, P], [1, CR * 24]], o24),
                )
                nc.vector.tensor_mul(
                    _v(lmu_ap, [[RPP * 24, P], [1, CR * 24]], o24),
                    _v(sq[:, :], [[RPP * 24, P], [1, CR * 24]], o24),
                    _v(cab_ap, [[48, P], [0, CR], [1, 24]], 24),
                )
                # lPP pair-log sums (feeds the TensorE groups)
                nc.vector.tensor_add(
                    _v(lpp_ap, [[RPP * 32, P], [32, CR], [4, 4], [1, 4]], o32),
                    _v(lmu_ap, [[RPP * 24, P], [24, CR], [1, 4], [0, 4]], o24 + 8),
                    _v(lmu_ap, [[RPP * 24, P], [24, CR], [0, 4], [1, 4]], o24 + 12),
                )
                nc.vector.tensor_add(
                    _v(lpp_ap, [[RPP * 32, P], [32, CR], [4, 4], [1, 4]], o32 + 16),
                    _v(lmu_ap, [[RPP * 24, P], [24, CR], [1, 4], [0, 4]], o24 + 16),
                    _v(lmu_ap, [[RPP * 24, P], [24, CR], [0, 4], [1, 4]], o24 + 20),
                )
                # mu = exp(lmu) (ACT, off the DVE critical path)
                nc.scalar.activation(
                    _v(mu_ap, [[RPP * 24, P], [1, CR * 24]], o24),
                    _v(lmu_ap, [[RPP * 24, P], [1, CR * 24]], o24),
                    AF.Exp,
                )

            # late constants: c2 cast and x_aug (needed by GMM / Q)
            c2 = cp.tile([P, 2 * NQ], BF16)
            nc.vector.tensor_copy(c2[:, :], c2f[:, :])
            nc.vector.tensor_copy(
                _v(xa_ap, [[RPP * NJ, P], [NJ, RPP], [1, NI]]),
                _v(xs_ap, [[RPP * NI, P], [NI, RPP], [1, NI]]),
            )
            nc.vector.memset(_v(xa_ap, [[RPP * NJ, P], [NJ, RPP], [1, 1]], 6), 1.0)

            for h in range(2):
                o24 = h * HR * 24
                o6 = h * HR * NI
                o7 = h * HR * NJ
                oA = h * HR * NA
                oQ = h * HR * NQ
                nc.vector.tensor_mul(
                    _v(p01_ap, [[RPP * NA, P], [NA, HR], [NT, NT], [1, NT]], oA),
                    _v(mu_ap, [[RPP * 24, P], [24, HR], [1, NT], [0, NT]], o24 + 0),
                    _v(mu_ap, [[RPP * 24, P], [24, HR], [0, NT], [1, NT]], o24 + 4),
                )
                # Q = P01 (x) x_aug
                nc.vector.tensor_mul(
                    _v(q[:, :], [[RPP * NQ, P], [NQ, HR], [NJ, NA], [1, NJ]], oQ),
                    _v(p01_ap, [[RPP * NA, P], [NA, HR], [1, NA], [0, NJ]], oA),
                    _v(xa_ap, [[RPP * NJ, P], [NJ, HR], [0, NA], [1, NJ]], o7),
                )
                # denominator partial: e1[p,r,i] = sum_t mu
                nc.vector.tensor_reduce(
                    _v(e1_ap, [[RPP * NI, P], [NI, HR], [1, NI]], o6),
                    _v(mu_ap, [[RPP * 24, P], [24, HR], [4, NI], [1, NT]], o24),
                    axis=AX.X,
                    op=ALU.add,
                )

            # ---------------- denominator: prod_i of the 6 sums ----------------
            p3 = cp.tile([P, RPP * 3], F32)
            nc.vector.tensor_mul(
                _v(p3[:, :], [[RPP * 3, P], [3, RPP], [1, 3]]),
                _v(e1_ap, [[RPP * NI, P], [NI, RPP], [1, 3]]),
                _v(e1_ap, [[RPP * NI, P], [NI, RPP], [1, 3]], 3),
            )
            p3_ap = p3[:, :]
            den1 = cp.tile([P, RPP], F32)
            den2 = cp.tile([P, RPP], F32)
            nc.vector.tensor_mul(
                den1[:, :],
                _v(p3_ap, [[RPP * 3, P], [3, RPP]], 0),
                _v(p3_ap, [[RPP * 3, P], [3, RPP]], 1),
            )
            nc.vector.tensor_mul(
                den2[:, :],
                den1[:, :],
                _v(p3_ap, [[RPP * 3, P], [3, RPP]], 2),
            )
            dene = cp.tile([P, RPP], F32)
            recip = cp.tile([P, RPP], F32)
            nc.vector.tensor_scalar_add(dene[:, :], den2[:, :], 1e-8)
            nc.vector.reciprocal(recip[:, :], dene[:, :])

            # ---------------- per packed-row pass (groups of 4 rows) ----------------
            num = cp.tile([P, RPP], F32)
            GR = 4  # rows per group
            for g in range(RPP // GR):
                # 4 transposes -> one [32, 4*128] psum tile: cols = (rr, b)
                ptt4 = ptt_pool.tile([32, GR * P], BF16)
                for rr in range(GR):
                    r = g * GR + rr
                    nc.tensor.transpose(
                        ptt4[:, rr * P : (rr + 1) * P],
                        lpp[:, r * 32 : (r + 1) * 32],
                        idn[:, :],
                    )
                lppT4 = lp_pool.tile([32, GR * P], BF16)
                nc.scalar.copy(lppT4[:, :], ptt4[:, :])

                # Kron-expansion matmuls, N = GR*128 covering GR rows at once
                NW = GR * P
                plw4 = plw_pool.tile([P, 2 * NW], F32)
                for ch in range(2):
                    nc.tensor.matmul(
                        plw4[:, ch * NW : ch * NW + NW],
                        rt[0:32, ch * 128 : (ch + 1) * 128],
                        lppT4[:, :],
                        start=True,
                        stop=True,
                    )
                p45g = p45_pool.tile([P, 2 * NW], BF16)
                for eh in range(2 * NW // 512):
                    nc.scalar.activation(
                        p45g[:, eh * 512 : (eh + 1) * 512],
                        plw4[:, eh * 512 : (eh + 1) * 512],
                        AF.Exp,
                    )

                pg4 = pg_pool.tile([P, GR * NQ], F32, tag="pg")
                for rr in range(GR):
                    nc.tensor.matmul(
                        pg4[:, rr * NQ : (rr + 1) * NQ],
                        p45g[:, rr * P : (rr + 1) * P],
                        c2[:, 0:NQ],
                        start=True,
                        stop=False,
                    )
                    nc.tensor.matmul(
                        pg4[:, rr * NQ : (rr + 1) * NQ],
                        p45g[:, NW + rr * P : NW + (rr + 1) * P],
                        c2[:, NQ : 2 * NQ],
                        start=False,
                        stop=True,
                    )
                for rr in range(GR):
                    r = g * GR + rr
                    scr = scr_pool.tile([P, NQ], F32)
                    nc.vector.scalar_tensor_tensor(
                        out=scr[:, :],
                        in0=pg4[:, rr * NQ : (rr + 1) * NQ],
                        scalar=1.0,
                        in1=q[:, r * NQ : (r + 1) * NQ],
                        op0=ALU.mult,
                        op1=ALU.mult,
                        accum_out=num[:, r : r + 1],
                    )

            # ---------------- out = num * recip ----------------
            res = cp.tile([P, RPP], F32)
            nc.vector.tensor_mul(res[:, :], num[:, :], recip[:, :])
            nc.sync.dma_start(
                o_e.ap().rearrange("(p r) o -> p (r o)", p=P), res[:, :]
            )

    return nc


def make_rt():
    rtm = np.zeros((32, NB), dtype=np.float32)
    for bm in range(NB):
        rtm[bm // 16, bm] = 1.0
        rtm[16 + bm % 16, bm] = 1.0
    return np.tile(rtm, (4, 1))  # replicated at partition bases 0/32/64/96


_CACHE = {}


def _get_nc():
    if "nc" not in _CACHE:
        _CACHE["nc"] = split_multi_waits(build_kernel())
    return _CACHE["nc"]


def make_in_maps(x, centers, sigmas, consequents):
    x = np.ascontiguousarray(x, dtype=np.float32)
    centers = np.ascontiguousarray(centers, dtype=np.float32)
    sigmas = np.ascontiguousarray(sigmas, dtype=np.float32)
    consequents = np.ascontiguousarray(consequents, dtype=np.float32)
    import ml_dtypes

    rtm = make_rt().astype(ml_dtypes.bfloat16)
    idn = np.eye(P, dtype=ml_dtypes.bfloat16)
    in_maps = []
    for i in range(NCORES):
        in_maps.append(
            {
                "x": x[i * BC : (i + 1) * BC],
                "centers": centers,
                "sigmas": sigmas,
                "consequents": consequents,
                "rt": rtm,
                "ident": idn,
            }
        )
    return in_maps


def kernel(x, centers, sigmas, consequents):
    nc = _get_nc()
    in_maps = make_in_maps(x, centers, sigmas, consequents)
    res = bass_utils.run_bass_kernel_spmd(nc, in_maps, core_ids=list(range(NCORES)))
    outs = [res.results[i]["out"] for i in range(NCORES)]
    return np.concatenate(outs, axis=0).astype(np.float32)


if __name__ == "__main__":
    rng = np.random.default_rng(0)
    x = rng.standard_normal((B_FULL, NI), dtype=np.float32)
    centers = rng.standard_normal((NI, NT), dtype=np.float32)
    sigmas = rng.uniform(0.5, 1.5, (NI, NT)).astype(np.float32)
    consequents = rng.standard_normal((4096, NJ), dtype=np.float32)
    out = kernel(x, centers, sigmas, consequents)
    print(out.shape, out.dtype, out[:4, 0])


# revision 33
# speedup vs baseline: 1.0141x; 1.0109x over previous
"""ANFIS layer kernel for 8 Trainium2 NeuronCores (data-parallel over batch).

Math (reference):
  mu[b,i,t]   = exp(-(x[b,i]-c[i,t])^2 / (2 s[i,t]^2)),  s = |sigmas|+1e-6
  w[b,r]      = prod_i mu[b,i,RULE_IDX[r,i]]   (RULE_IDX = full cross product)
  out[b]      = sum_r w[b,r] f[b,r] / (sum_r w[b,r] + 1e-8),  f = [x,1] @ C^T

Factorization used here (r = A*256 + B, A=(t0,t1) in 16, B=(t2..t5) in 256):
  sum_r w[b,r]        = prod_i sum_t mu[b,i,t]                (product of sums)
  sum_r w[b,r] f[b,r] = sum_{A,B,j} P01[b,A] P2345[b,B] xa[b,j] C[A,B,j]
                      = sum_{A,j} (P01 (x) xa)[b,(A,j)] * (P2345[b,:] @ C2)[b,(A,j)]
  with C2[B,(A,j)] = consequents[A*256+B, j].

Device mapping per core (4096 batch rows, packed 32 rows per partition):
  - log-memberships lmu = -(x-c)^2/(2 s^2) built with packed vector ops
  - P2345[b,B] = exp(lP23[b,B//16] + lP45[b,B%16]) computed on TensorE:
    transpose the 32 stacked log-pair columns, matmul with a 0/1
    replication matrix RT[32,256], exp on ScalarE (PSUM->SBUF)
  - G = P2345 @ C2 via 2 PSUM-accumulating matmuls (K=128 each)
  - numerator via fused multiply-reduce against Q = P01 (x) x_aug
"""

import os
import sys

import numpy as np

if "/opt/trn_rl_repo" not in sys.path:
    sys.path.insert(0, "/opt/trn_rl_repo")

import concourse.bass as bass
import concourse.mybir as mybir
from concourse import tile
from concourse import bass_utils

F32 = mybir.dt.float32
BF16 = mybir.dt.bfloat16
AF = mybir.ActivationFunctionType
ALU = mybir.AluOpType
AX = mybir.AxisListType

NCORES = 8
B_FULL = 32768
BC = B_FULL // NCORES  # 4096 rows per core
P = 128                # partitions
RPP = BC // P          # 32 batch rows packed per partition
NI = 6                 # inputs
NT = 4                 # terms per input
NA = 16                # left modes  (t0,t1)
NB = 256               # right modes (t2..t5)
NJ = NI + 1            # augmented input width
NQ = NA * NJ           # 112


def _v(ap, ap_list, extra_offset=0):
    """Build a raw AP view with an explicit [step, count] list."""
    return bass.AP(ap.tensor, ap.offset + extra_offset, [list(d) for d in ap_list])


def split_multi_waits(nc):
    """TRN2 instructions hold at most one sync-wait; walrus rejects more.

    Tile emits multi-wait instructions, so hoist all but the last wait into
    single-wait EventSemaphore instructions on the same engine (engine
    program order preserves semantics).
    """
    for f in nc.m.functions:
        for blk in f.blocks:
            i = 0
            while i < len(blk.instructions):
                inst = blk.instructions[i]
                si = inst.sync_info
                if (
                    si is not None
                    and si.on_wait
                    and len(si.on_wait) > 1
                    and inst.engine != mybir.EngineType.Unassigned
                ):
                    waits = list(si.on_wait)
                    for w in waits[:-1]:
                        ev = mybir.InstEventSemaphore(
                            name=nc.get_next_instruction_name(), ins=[], outs=[]
                        )
                        ev.engine = inst.engine
                        ev.sync_info = mybir.SyncInfo(on_wait=[w], on_update=[])
                        nc.register_instruction(ev)
                        blk.instructions.insert(i, ev)
                        i += 1
                    inst.sync_info = mybir.SyncInfo(
                        on_wait=[waits[-1]], on_update=list(si.on_update)
                    )
                i += 1
    return nc


def build_kernel():
    nc = bass.Bass(target_bir_lowering=False)

    x_e = nc.declare_dram_parameter("x", [BC, NI], F32, isOutput=False)
    c_e = nc.declare_dram_parameter("centers", [NI, NT], F32, isOutput=False)
    s_e = nc.declare_dram_parameter("sigmas", [NI, NT], F32, isOutput=False)
    q_e = nc.declare_dram_parameter("consequents", [4096, NJ], F32, isOutput=False)
    rt_e = nc.declare_dram_parameter("rt", [P, NB], BF16, isOutput=False)
    id_e = nc.declare_dram_parameter("ident", [P, P], BF16, isOutput=False)
    o_e = nc.declare_dram_parameter("out", [BC, 1], F32, isOutput=True)

    with tile.TileContext(nc) as tc:
        with (
            tc.tile_pool(name="const", bufs=1) as cp,
            tc.tile_pool(name="lppT", bufs=6) as lp_pool,
            tc.tile_pool(name="p45", bufs=4) as p45_pool,
            tc.tile_pool(name="scr", bufs=6) as scr_pool,
            tc.tile_pool(name="ptt", bufs=2, space="PSUM") as ptt_pool,
            tc.tile_pool(name="plw", bufs=2, space="PSUM") as plw_pool,
            tc.tile_pool(name="pg", bufs=2, space="PSUM") as pg_pool,
        ):
            # ---------------- inputs -> SBUF ----------------
            # params first (the membership chain needs them), broadcast to all
            # 128 partitions with a 0-step source AP; then x; then the
            # later-needed constants.
            cab = cp.tile([P, 48], F32)  # [cB(24) | aB(24)] replicated
            sB = cp.tile([P, 24], F32)
            nc.sync.dma_start(
                _v(cab[:, :], [[48, P], [1, 24]]), _v(c_e.ap(), [[0, P], [1, 24]])
            )
            nc.sync.dma_start(
                _v(sB[:, :], [[24, P], [1, 24]]), _v(s_e.ap(), [[0, P], [1, 24]])
            )
            xs = cp.tile([P, RPP * NI], F32)  # x packed: [p, r, i]
            nc.sync.dma_start(xs[:, :], x_e.ap().rearrange("(p r) i -> p (r i)", p=P))

            idn = cp.tile([P, P], BF16)
            nc.sync.dma_start(idn[:, :], id_e[:, :])
            rt = cp.tile([P, NB], BF16)
            nc.sync.dma_start(rt[:, :], rt_e[:, :])

            # C2[B, (A,j)] = consequents[A*256+B, j]; chunk k holds B in [128k, 128k+128)
            c2f = cp.tile([P, 2 * NQ], F32)
            cap = [[NJ, P], [NB * NJ, NA], [1, NJ]]
            nc.sync.dma_start(c2f[:, 0:NQ], _v(q_e.ap(), cap))
            nc.sync.dma_start(c2f[:, NQ : 2 * NQ], _v(q_e.ap(), cap, extra_offset=P * NJ))
            # aB = -1/(2 s^2)  (sigmas in [0.5, 1.5]: (|s|+1e-6)^2 ~= s^2)
            cab_ap = cab[:, :]
            nc.vector.tensor_mul(sB[:, :], sB[:, :], sB[:, :])
            nc.vector.reciprocal(_v(cab_ap, [[48, P], [1, 24]], 24), sB[:, :])
            nc.vector.tensor_scalar_mul(
                _v(cab_ap, [[48, P], [1, 24]], 24),
                _v(cab_ap, [[48, P], [1, 24]], 24),
                -0.5,
            )
            # cB = cab[:, 0:24] ; aB = cab[:, 24:48]

            # ---------------- membership prologue, split in row-halves so the
            # TensorEngine groups can start while the second half computes.
            # The lpp chain is emitted first (it gates the TensorE groups);
            # mu/P01/Q/e1 follow once both halves' logs are in flight. ----
            d = cp.tile([P, RPP * 24], F32)
            sq = cp.tile([P, RPP * 24], F32)
            lmu = cp.tile([P, RPP * 24], F32)
            mu = cp.tile([P, RPP * 24], F32)
            lpp = cp.tile([P, RPP * 32], BF16)
            p01 = cp.tile([P, RPP * NA], F32)
            q = cp.tile([P, RPP * NQ], BF16)
            e1 = cp.tile([P, RPP * NI], F32)
            xa = cp.tile([P, RPP * NJ], F32)
            d_ap = d[:, :]
            mu_ap = mu[:, :]
            lpp_ap = lpp[:, :]
            lmu_ap = lmu[:, :]
            p01_ap = p01[:, :]
            e1_ap = e1[:, :]
            xa_ap = xa[:, :]
            xs_ap = xs[:, :]
            HR = RPP // 2  # rows per half
            NCH = 4        # chunks for the log chain (first groups start early)
            CR = RPP // NCH
            for h in range(NCH):
                o24 = h * CR * 24
                o32 = h * CR * 32
                o6 = h * CR * NI
                # d[p,r,i,t] = x[p,r,i] - c[i,t] ; lmu = -(d*d) / (2 s^2)
                nc.vector.tensor_sub(
                    _v(d_ap, [[RPP * 24, P], [24, CR], [4, NI], [1, NT]], o24),
                    _v(xs_ap, [[RPP * NI, P], [NI, CR], [1, NI], [0, NT]], o6),
                    _v(cab_ap, [[48, P], [0, CR], [4, NI], [1, NT]]),
                )
                nc.vector.tensor_mul(
                    _v(sq[:, :], [[RPP * 24, P], [1, CR * 24]], o24),
                    _v(d_ap, [[RPP * 24, P], [1, CR * 24]], o24),
                    _v(d_ap, [[RPP * 24, P], [1, CR * 24]], o24),
                )
                nc.vector.tensor_mul(
                    _v(lmu_ap, [[RPP * 24, P], [1, CR * 24]], o24),
                    _v(sq[:, :], [[RPP * 24, P], [1, CR * 24]], o24),
                    _v(cab_ap, [[48, P], [0, CR], [1, 24]], 24),
                )
                # lPP pair-log sums (feeds the TensorE groups)
                nc.vector.tensor_add(
                    _v(lpp_ap, [[RPP * 32, P], [32, CR], [4, 4], [1, 4]], o32),
                    _v(lmu_ap, [[RPP * 24, P], [24, CR], [1, 4], [0, 4]], o24 + 8),
                    _v(lmu_ap, [[RPP * 24, P], [24, CR], [0, 4], [1, 4]], o24 + 12),
                )
                nc.vector.tensor_add(
                    _v(lpp_ap, [[RPP * 32, P], [32, CR], [4, 4], [1, 4]], o32 + 16),
                    _v(lmu_ap, [[RPP * 24, P], [24, CR], [1, 4], [0, 4]], o24 + 16),
                    _v(lmu_ap, [[RPP * 24, P], [24, CR], [0, 4], [1, 4]], o24 + 20),
                )
                # mu = exp(lmu) (ACT, off the DVE critical path)
                nc.scalar.activation(
                    _v(mu_ap, [[RPP * 24, P], [1, CR * 24]], o24),
                    _v(lmu_ap, [[RPP * 24, P], [1, CR * 24]], o24),
                    AF.Exp,
                )

            # late constants: c2 cast and x_aug (needed by GMM / Q)
            c2 = cp.tile([P, 2 * NQ], BF16)
            nc.vector.tensor_copy(c2[:, :], c2f[:, :])
            nc.vector.tensor_copy(
                _v(xa_ap, [[RPP * NJ, P], [NJ, RPP], [1, NI]]),
                _v(xs_ap, [[RPP * NI, P], [NI, RPP], [1, NI]]),
            )
            nc.vector.memset(_v(xa_ap, [[RPP * NJ, P], [NJ, RPP], [1, 1]], 6), 1.0)

            for h in range(2):
                o24 = h * HR * 24
                o6 = h * HR * NI
                o7 = h * HR * NJ
                oA = h * HR * NA
                oQ = h * HR * NQ
                nc.vector.tensor_mul(
                    _v(p01_ap, [[RPP * NA, P], [NA, HR], [NT, NT], [1, NT]], oA),
                    _v(mu_ap, [[RPP * 24, P], [24, HR], [1, NT], [0, NT]], o24 + 0),
                    _v(mu_ap, [[RPP * 24, P], [24, HR], [0, NT], [1, NT]], o24 + 4),
                )
                # Q = P01 (x) x_aug
                nc.vector.tensor_mul(
                    _v(q[:, :], [[RPP * NQ, P], [NQ, HR], [NJ, NA], [1, NJ]], oQ),
                    _v(p01_ap, [[RPP * NA, P], [NA, HR], [1, NA], [0, NJ]], oA),
                    _v(xa_ap, [[RPP * NJ, P], [NJ, HR], [0, NA], [1, NJ]], o7),
                )
                # denominator partial: e1[p,r,i] = sum_t mu
                nc.vector.tensor_reduce(
                    _v(e1_ap, [[RPP * NI, P], [NI, HR], [1, NI]], o6),
                    _v(mu_ap, [[RPP * 24, P], [24, HR], [4, NI], [1, NT]], o24),
                    axis=AX.X,
                    op=ALU.add,
                )

            # ---------------- denominator: prod_i of the 6 sums ----------------
            p3 = cp.tile([P, RPP * 3], F32)
            nc.vector.tensor_mul(
                _v(p3[:, :], [[RPP * 3, P], [3, RPP], [1, 3]]),
                _v(e1_ap, [[RPP * NI, P], [NI, RPP], [1, 3]]),
                _v(e1_ap, [[RPP * NI, P], [NI, RPP], [1, 3]], 3),
            )
            p3_ap = p3[:, :]
            den1 = cp.tile([P, RPP], F32)
            den2 = cp.tile([P, RPP], F32)
            nc.vector.tensor_mul(
                den1[:, :],
                _v(p3_ap, [[RPP * 3, P], [3, RPP]], 0),
                _v(p3_ap, [[RPP * 3, P], [3, RPP]], 1),
            )
            nc.vector.tensor_mul(
                den2[:, :],
                den1[:, :],
                _v(p3_ap, [[RPP * 3, P], [3, RPP]], 2),
            )
            dene = cp.tile([P, RPP], F32)
            recip = cp.tile([P, RPP], F32)
            nc.vector.tensor_scalar_add(dene[:, :], den2[:, :], 1e-8)
            nc.vector.reciprocal(recip[:, :], dene[:, :])

            # ---------------- per packed-row pass (groups of 4 rows) ----------------
            num = cp.tile([P, RPP], F32)
            GR = 4  # rows per group
            for g in range(RPP // GR):
                # 4 transposes -> one [32, 4*128] psum tile: cols = (rr, b)
                ptt4 = ptt_pool.tile([32, GR * P], BF16)
                for rr in range(GR):
                    r = g * GR + rr
                    nc.tensor.transpose(
                        ptt4[:, rr * P : (rr + 1) * P],
                        lpp[:, r * 32 : (r + 1) * 32],
                        idn[:, :],
                    )
                lppT4 = lp_pool.tile([32, GR * P], BF16)
                nc.scalar.copy(lppT4[:, :], ptt4[:, :])

                # Kron-expansion matmuls, N = GR*128 covering GR rows at once
                NW = GR * P
                plw4 = plw_pool.tile([P, 2 * NW], F32)
                for ch in range(2):
                    nc.tensor.matmul(
                        plw4[:, ch * NW : ch * NW + NW],
                        rt[0:32, ch * 128 : (ch + 1) * 128],
                        lppT4[:, :],
                        start=True,
                        stop=True,
                    )
                p45g = p45_pool.tile([P, 2 * NW], BF16)
                for eh in range(2 * NW // 512):
                    nc.scalar.activation(
                        p45g[:, eh * 512 : (eh + 1) * 512],
                        plw4[:, eh * 512 : (eh + 1) * 512],
                        AF.Exp,
                    )

                pg4 = pg_pool.tile([P, GR * NQ], F32, tag="pg")
                for rr in range(GR):
                    nc.tensor.matmul(
                        pg4[:, rr * NQ : (rr + 1) * NQ],
                        p45g[:, rr * P : (rr + 1) * P],
                        c2[:, 0:NQ],
                        start=True,
                        stop=False,
                    )
                    nc.tensor.matmul(
                        pg4[:, rr * NQ : (rr + 1) * NQ],
                        p45g[:, NW + rr * P : NW + (rr + 1) * P],
                        c2[:, NQ : 2 * NQ],
                        start=False,
                        stop=True,
                    )
                for rr in range(GR):
                    r = g * GR + rr
                    scr = scr_pool.tile([P, NQ], F32)
                    nc.vector.scalar_tensor_tensor(
                        out=scr[:, :],
                        in0=pg4[:, rr * NQ : (rr + 1) * NQ],
                        scalar=1.0,
                        in1=q[:, r * NQ : (r + 1) * NQ],
                        op0=ALU.mult,
                        op1=ALU.mult,
                        accum_out=num[:, r : r + 1],
                    )

            # ---------------- out = num * recip ----------------
            res = cp.tile([P, RPP], F32)
            nc.vector.tensor_mul(res[:, :], num[:, :], recip[:, :])
            nc.sync.dma_start(
                o_e.ap().rearrange("(p r) o -> p (r o)", p=P), res[:, :]
            )

    return nc


def make_rt():
    rtm = np.zeros((32, NB), dtype=np.float32)
    for bm in range(NB):
        rtm[bm // 16, bm] = 1.0
        rtm[16 + bm % 16, bm] = 1.0
    return np.tile(rtm, (4, 1))  # replicated at partition bases 0/32/64/96


_CACHE = {}


def _get_nc():
    if "nc" not in _CACHE:
        _CACHE["nc"] = split_multi_waits(build_kernel())
    return _CACHE["nc"]


def make_in_maps(x, centers, sigmas, consequents):
    x = np.ascontiguousarray(x, dtype=np.float32)
    centers = np.ascontiguousarray(centers, dtype=np.float32)
    sigmas = np.ascontiguousarray(sigmas, dtype=np.float32)
    consequents = np.ascontiguousarray(consequents, dtype=np.float32)
    import ml_dtypes

    rtm = make_rt().astype(ml_dtypes.bfloat16)
    idn = np.eye(P, dtype=ml_dtypes.bfloat16)
    in_maps = []
    for i in range(NCORES):
        in_maps.append(
            {
                "x": x[i * BC : (i + 1) * BC],
                "centers": centers,
                "sigmas": sigmas,
                "consequents": consequents,
                "rt": rtm,
                "ident": idn,
            }
        )
    return in_maps


def kernel(x, centers, sigmas, consequents):
    nc = _get_nc()
    in_maps = make_in_maps(x, centers, sigmas, consequents)
    res = bass_utils.run_bass_kernel_spmd(nc, in_maps, core_ids=list(range(NCORES)))
    outs = [res.results[i]["out"] for i in range(NCORES)]
    return np.concatenate(outs, axis=0).astype(np.float32)


if __name__ == "__main__":
    rng = np.random.default_rng(0)
    x = rng.standard_normal((B_FULL, NI), dtype=np.float32)
    centers = rng.standard_normal((NI, NT), dtype=np.float32)
    sigmas = rng.uniform(0.5, 1.5, (NI, NT)).astype(np.float32)
    consequents = rng.standard_normal((4096, NJ), dtype=np.float32)
    out = kernel(x, centers, sigmas, consequents)
    print(out.shape, out.dtype, out[:4, 0])
